# revision 52
# baseline (speedup 1.0000x reference)
"""Trainium2 Bass kernel for nn_DA_conv: per-sample dynamic depthwise 3x3 conv
(+LeakyReLU) followed by a 1x1 pointwise conv, with the 3x3 kernels produced by
a small per-sample MLP.

Strategy (8 NeuronCores, pure batch data-parallel, 2 samples per core).

CoreSim cost model facts this design is built around (measured):
  - matmul charges output-free-size x 0.417ns regardless of partition count
    or tile_position, so every matmul uses all 128 partitions (2 samples x 64
    channels) via diagonal / block-diagonal weights.
  - DVE tensor_scalar runs at 4x for 2-byte SBUF operands (194ns/512),
    tensor_tensor at 2x (327ns/512), scalar_tensor_tensor gets no speedup.
  - Pool runs TS/TT/STT flat at 0.833ns/elem with no per-op init and may
    read-modify-write PSUM (853ns/1024).
  - Act activation costs ~(N*0.833 + 143..370)ns; Prelu alpha=1.0 is an
    identity copy on the same table (no act-table reload).
  - DMA is one serialized resource; contiguous chunks >=512B get 360GB/s,
    smaller chunks pay 2x.

So the 9 depthwise taps + the rest are split across engines (per-image
totals): PE 5 taps + 1x1 (41us), Pool 2 taps + partial->PSUM merge (41us),
DVE 2 taps + pair-merge + half output evac (30us), Act lrelu + half output
evac (26us), DMA fp16-in/fp16-out (23us).

The feature map lives in SBUF fp16 with rows contiguous (stride 128 = DRAM
layout, full-rate DMA) plus one zero row above/below. Horizontal SAME-padding
is realized by TRIMMING the access patterns of the dj!=1 taps (the border
column simply doesn't receive that tap) instead of padding columns, which
keeps every DMA contiguous.
"""

import os
import sys

sys.path.insert(0, "/opt/trn_rl_repo")

from contextlib import ExitStack

import numpy as np

import concourse.bacc as bacc
import concourse.bass as bass
import concourse.mybir as mybir
import concourse.tile as tile

S = 2            # samples per core
C = 64           # channels
H = W = 128      # spatial
KK = 3           # conv kernel size
NCORES = 8
RP = H + 2       # stored rows (zero halo row above and below)
SR = 8           # image rows per super-tile
NST = H // SR    # 16 super-tiles
SPX = SR * W     # 1024 pixels per super-tile (2 PSUM banks)

f32 = mybir.dt.float32
f16 = mybir.dt.float16
i32 = mybir.dt.int32

X_MODE = "f16"   # kept for test.py compatibility

LRELU = mybir.ActivationFunctionType.Prelu

# tap split across engines: (di, dj) with di=vertical, dj=horizontal.
# GPSIMD/Pool supports no generic elementwise ops on HW, so PE and DVE carry
# everything, balanced at ~49us each. DVE takes 4 taps: the two full-width
# center-column taps plus the (0,0)/(2,0) pair -- both dj=0, so their
# x-aligned full-width tensor_scalar products (4x DVE mode) merge with one
# full tensor_tensor and one column-trimmed tensor_tensor. PE carries the
# other 5 taps as diagonal matmuls, adds v into PSUM with an identity matmul
# inside the accumulation group, and runs the 1x1.
PE_TAPS = [(1, 1), (0, 2), (2, 2), (1, 0), (1, 2)]   # (1,1) first: start=True
DVE_TAPS = [(0, 1), (2, 1)]                          # full-width center column
DVE_PAIR = [(0, 0), (2, 0)]                          # dj=0 pair, trimmed merge


def _cols(dj):
    """(in_col_slice, out_col_slice) realizing horizontal SAME padding."""
    if dj == 0:
        return slice(0, W - 1), slice(1, W)
    if dj == 1:
        return slice(0, W), slice(0, W)
    return slice(1, W), slice(0, W - 1)


def build_program() -> bass.Bass:
    nc = bacc.Bacc("TRN2", target_bir_lowering=False, debug=False)

    x_d = nc.dram_tensor("x", [S * C, H * W], f16, kind="ExternalInput").ap()
    # d2[s*64+i] = d[s, i] (both samples stacked in one column)
    d2_d = nc.dram_tensor("d2", [S * C, 1], f32, kind="ExternalInput").ap()
    # block-diag(Wk1.T, Wk1.T)
    wk1_d = nc.dram_tensor("wk1b", [S * C, S * C], f32,
                           kind="ExternalInput").ap()
    # per-tap block-diag(Wk2_t.T, Wk2_t.T): wk2b[j2, t*128 + p]
    wk2_d = nc.dram_tensor("wk2b", [S * C, KK * KK * S * C], f16,
                           kind="ExternalInput").ap()
    # block-diag(Wc.T, Wc.T) in fp16
    wc2_d = nc.dram_tensor("wc2", [S * C, S * C], f16, kind="ExternalInput").ap()
    out_d = nc.dram_tensor("out", [S * C, H * W], f16, kind="ExternalOutput").ap()

    with tile.TileContext(nc) as tc, ExitStack() as ctx:
        _body(ctx, tc, x_d, d2_d, wk1_d, wk2_d, wc2_d, out_d)
    nc.compile()
    return nc


def _body(ctx, tc, x_d, d2_d, wk1_d, wk2_d, wc2_d, out_d):
    nc = tc.nc
    P128 = S * C
    const = ctx.enter_context(tc.tile_pool(name="const", bufs=1))
    xpool = ctx.enter_context(tc.tile_pool(name="xs", bufs=1))
    vpool = ctx.enter_context(tc.tile_pool(name="v", bufs=3))
    apool = ctx.enter_context(tc.tile_pool(name="a", bufs=3))
    zpool = ctx.enter_context(tc.tile_pool(name="z", bufs=2))
    pdw = ctx.enter_context(tc.tile_pool(name="pdw", bufs=2, space="PSUM"))
    po = ctx.enter_context(tc.tile_pool(name="po", bufs=4, space="PSUM"))

    # Preload the Prelu activation table while the weight DMAs are in
    # flight; otherwise the 1283ns table load lands on the MLP critical path.
    warm = const.tile([C, 1], f32)
    nc.vector.memset(warm[:, :], 0.0)
    nc.scalar.activation(warm[:, :], warm[:, :], LRELU, alpha=0.1)

    # ---------------- small-weight loads (MLP deps first) ----------------
    wk1b = const.tile([P128, P128], f32)
    nc.sync.dma_start(wk1b[:, :], wk1_d)
    d2s = const.tile([P128, 1], f32)
    nc.sync.dma_start(d2s[:, :], d2_d)
    # ---------------- resident feature map (fp16, contiguous rows) ----------
    # first two x chunks jump the DMA queue ahead of the big weights so the
    # DVE tap products can start as early as possible
    xs = xpool.tile([P128, RP * W], f16)
    nc.vector.memset(xs[:, 0:W], 0.0)
    nc.vector.memset(xs[:, (RP - 1) * W : RP * W], 0.0)

    def xchunk(k):
        dst = xs[:, (1 + k * SR) * W : (1 + (k + 1) * SR) * W]
        nc.sync.dma_start(dst, x_d[:, k * SPX : (k + 1) * SPX])

    wk2b = const.tile([P128, KK * KK * P128], f16)
    nc.sync.dma_start(wk2b[:, :], wk2_d)
    for k in range(2):
        xchunk(k)
    wc2 = const.tile([P128, P128], f16)
    nc.sync.dma_start(wc2[:, :], wc2_d)
    for k in range(2, NST):
        xchunk(k)

    # ---------------- kernel-generating MLP ----------------
    hid_ps = po.tile([P128, 1], f32, tag="o")
    nc.tensor.matmul(hid_ps[:, :], lhsT=wk1b[:, :], rhs=d2s[:, :],
                     start=True, stop=True)
    # lrelu(x) = max(x, 0.1x) on DVE -- keeps the MLP critical path off the
    # Activation engine (whose queue holds the act-table preload).
    hid_t = const.tile([P128, 1], f32)
    nc.vector.tensor_scalar_mul(hid_t[:, :], hid_ps[:, :], 0.1)
    hid_sb = const.tile([P128, 1], f16)
    nc.vector.tensor_tensor(hid_sb[:, :], hid_ps[:, :], hid_t[:, :],
                            op=mybir.AluOpType.max)

    # identity matrix (independent of the MLP; emitted early)
    id_i = const.tile([P128, P128], i32)
    nc.gpsimd.iota(id_i[:, :], pattern=[[1, P128]], base=0,
                   channel_multiplier=-1)
    idf = const.tile([P128, P128], f16)
    nc.vector.tensor_scalar(idf[:, :], id_i[:, :], 0, None,
                            mybir.AluOpType.is_equal)

    # kcols[s*64+c, t] = kern[s, c*9+t]  (fp32, per-partition tap scalars) and
    # fp16 diagonal weight matrices, built in PE-tap order so the first
    # depthwise matmul can issue as early as possible.
    # each tap's block-diag Wk2 matmul produces kern for all 128 (s,c) pairs
    # as one [128,1] column; the PSUM->SBUF copies run on the (idle) Act
    # engine so DVE can start tap products immediately.
    ntap = KK * KK
    kcols = const.tile([P128, ntap], f32)
    diag = const.tile([P128, ntap * P128], f16)
    tap_order = [PE_TAPS[0]] + DVE_TAPS + DVE_PAIR + PE_TAPS[1:]
    for (di, dj) in tap_order:
        t = di * KK + dj
        kp = po.tile([P128, 1], f32, tag="o")
        nc.tensor.matmul(kp[:, :], lhsT=wk2b[:, t * P128 : (t + 1) * P128],
                         rhs=hid_sb[:, :], start=True, stop=True)
        if (di, dj) in PE_TAPS:
            nc.scalar.activation(kcols[:, t : t + 1], kp[:, :],
                                 mybir.ActivationFunctionType.Copy)
            nc.vector.tensor_scalar_mul(diag[:, t * P128 : (t + 1) * P128],
                                        idf[:, :], kcols[:, t : t + 1])
        else:
            nc.vector.tensor_copy(kcols[:, t : t + 1], kp[:, :])

    # ---------------- main loop ----------------
    xr = xs[:, :].rearrange("p (r w) -> p r w", w=W)

    def kap(di, dj):
        return kcols[:, (di * KK + dj) : (di * KK + dj) + 1]

    def dve_products(k0, nt):
        # tap products for nt consecutive super-tiles in one go -- larger TS
        # ops amortize the per-op DVE init (4x DVE mode). All four products
        # are full-width x-aligned.
        parts = []
        for i, (di, dj) in enumerate(DVE_TAPS + DVE_PAIR):
            t = vpool.tile([P128, nt * SPX], f16, tag=f"t{i}")
            w = xr[:, SR * k0 + di : SR * k0 + di + nt * SR, :]
            nc.vector.tensor_scalar_mul(t[:, :], w, kap(di, dj))
            parts.append(t)
        return parts

    def dve_merge(k, parts, off):
        # per-tile TT merges keep v latency tile-local; the dj=0 pair merges
        # via a column-trimmed TT, realizing SAME padding (col 0 gets no
        # dj=0 contribution).
        t0, t1, t2, t3 = parts
        sl = slice(off, off + SPX)
        v = vpool.tile([P128, SPX], f16, tag="v")
        nc.vector.tensor_tensor(v[:, :], t0[:, sl], t1[:, sl],
                                op=mybir.AluOpType.add)
        nc.vector.tensor_tensor(t2[:, sl], t2[:, sl], t3[:, sl],
                                op=mybir.AluOpType.add)
        vv = v[:, :].rearrange("p (r w) -> p r w", w=W)
        qv = t2[:, sl].rearrange("p (r w) -> p r w", w=W)
        nc.vector.tensor_tensor(vv[:, :, 1:W], vv[:, :, 1:W],
                                qv[:, :, 0 : W - 1], op=mybir.AluOpType.add)
        return v

    def pe_taps_half(k, P, h):
        Pv = P[:, :].rearrange("p (r w) -> p r w", w=W)
        out = Pv[:, 4 * h : 4 * h + 4, :]
        r0 = SR * k + 4 * h
        for i, (di, dj) in enumerate(PE_TAPS):
            ci, co = _cols(dj)
            nc.tensor.matmul(
                out[:, :, co],
                lhsT=diag[:, (di * KK + dj) * P128 : (di * KK + dj + 1) * P128],
                rhs=xr[:, r0 + di : r0 + di + 4, ci],
                start=(i == 0), stop=False,
                skip_group_check=True)

    def pe_iadd_half(k, P, v, h):
        # identity matmul folds the DVE partial into the accumulation group
        # (stop=True): no cross-engine PSUM merge needed.
        nc.tensor.matmul(
            P[:, 512 * h : 512 * (h + 1)], lhsT=idf[:, :],
            rhs=v[:, 512 * h : 512 * (h + 1)],
            start=False, stop=True, skip_group_check=True)

    def pe_taps(k):
        P = pdw.tile([P128, SPX], f32, tag="p")
        for h in (0, 1):
            pe_taps_half(k, P, h)
        return P

    def pe_iadd(k, P, v):
        for h in (0, 1):
            pe_iadd_half(k, P, v, h)

    def prelu_stage(k, P):
        # Full-width in steady state; half granularity in the last tiles
        # keeps the pipeline drain short.
        halves = k >= NST - 2
        a = apool.tile([P128, SPX], f16, tag="a")
        for hs in ([slice(0, 512), slice(512, SPX)] if halves
                   else [slice(0, SPX)]):
            nc.scalar.activation(a[:, hs], P[:, hs], LRELU, alpha=0.1)
        return a

    def conv1x1_half(k, h, a):
        O = po.tile([P128, 512], f32, tag="o")
        nc.tensor.matmul(O[:, :], lhsT=wc2[:, :], rhs=a[:, 512 * h : 512 * (h + 1)],
                         start=True, stop=True)
        z = zpool.tile([P128, 512], f16, tag=f"z{h}")
        if k >= NST - 1:
            # Act is congested with prelu halves during the drain; DVE idles
            nc.vector.tensor_copy(z[:, :], O[:, :])
        else:
            nc.scalar.activation(z[:, :], O[:, :], LRELU, alpha=1.0)
        nc.sync.dma_start(out_d[:, k * SPX + 512 * h : k * SPX + 512 * (h + 1)],
                          z[:, :])

    # DVE product granularity: single tiles at the pipeline ends (short
    # v latency for fill/drain), 4-tile blocks in the middle (lower DVE op
    # overhead)
    blocks = [(0, 1), (1, 1), (2, 4), (6, 4), (10, 4),
              (NST - 2, 1), (NST - 1, 1)]
    tiles = []
    for (k0, nt) in blocks:
        tiles.extend((k0, nt, i) for i in range(nt))

    pending = []
    parts = None
    for (k0, nt, i) in tiles:
        k = k0 + i
        if i == 0:
            parts = dve_products(k0, nt)
        v = dve_merge(k, parts, SPX * i)
        if k == NST - 1:
            # last tile: interleave per half so the drain chain starts at
            # the tile's midpoint
            P = pdw.tile([P128, SPX], f32, tag="p")
            a = apool.tile([P128, SPX], f16, tag="a")
            for h in (0, 1):
                pe_taps_half(k, P, h)
                pe_iadd_half(k, P, v, h)
                nc.scalar.activation(a[:, 512 * h : 512 * (h + 1)],
                                     P[:, 512 * h : 512 * (h + 1)],
                                     LRELU, alpha=0.1)
        else:
            P = pe_taps(k)
            pe_iadd(k, P, v)
            a = prelu_stage(k, P)
        for item in pending:
            conv1x1_half(*item)
        pending = [(k, 0, a), (k, 1, a)]
    for item in pending:
        conv1x1_half(*item)


# ---------------------------------------------------------------------------
# host-side entry point
# ---------------------------------------------------------------------------

_PROGRAM_CACHE: dict[str, bass.Bass] = {}


def _get_program(x_mode: str = X_MODE) -> bass.Bass:
    if x_mode not in _PROGRAM_CACHE:
        _PROGRAM_CACHE[x_mode] = build_program()
    return _PROGRAM_CACHE[x_mode]


def _host_prep(inputs: dict, x_mode: str = X_MODE):
    x = np.asarray(inputs["x"], dtype=np.float32)
    d = np.asarray(inputs["d"], dtype=np.float32)
    Wk1 = np.asarray(inputs["Wk1"], dtype=np.float32)
    Wk2 = np.asarray(inputs["Wk2"], dtype=np.float32)
    Wc = np.asarray(inputs["Wc"], dtype=np.float32)

    wk1b = np.zeros((S * C, S * C), dtype=np.float32)
    wk1b[0:C, 0:C] = Wk1.T
    wk1b[C:, C:] = Wk1.T

    # wk2b[:, t*128:(t+1)*128] = block-diag(Wk2_t.T, Wk2_t.T),
    # where Wk2_t[c, j] = Wk2[c*9+t, j]
    w = Wk2.reshape(C, KK * KK, C)                    # (c, t, j)
    wk2b = np.zeros((S * C, KK * KK, S * C), dtype=np.float16)
    wk2b[0:C, :, 0:C] = w.transpose(2, 1, 0)          # (j, t, c)
    wk2b[C:, :, C:] = w.transpose(2, 1, 0)
    wk2b = np.ascontiguousarray(
        wk2b.transpose(0, 1, 2).reshape(S * C, KK * KK * S * C)
    )

    wc2 = np.zeros((S * C, S * C), dtype=np.float16)
    wc2[0:C, 0:C] = Wc.T
    wc2[C:, C:] = Wc.T

    xcast = x.astype(np.float16)

    in_maps = []
    for i in range(NCORES):
        xi = np.ascontiguousarray(xcast[S * i : S * (i + 1)].reshape(S * C, H * W))
        d2 = np.ascontiguousarray(
            d[S * i : S * (i + 1)].reshape(S * C, 1)
        )
        in_maps.append(
            {"x": xi, "d2": d2, "wk1b": wk1b, "wk2b": wk2b, "wc2": wc2}
        )
    return in_maps


def run_on_hw(inputs: dict, x_mode: str = X_MODE, **kwargs):
    """Run the SPMD kernel on 8 NeuronCores; returns (output, results)."""
    from concourse.bass_utils import run_bass_kernel_spmd

    nc = _get_program(x_mode)
    in_maps = _host_prep(inputs, x_mode)
    res = run_bass_kernel_spmd(nc, in_maps, core_ids=list(range(NCORES)), **kwargs)
    outs = res.results
    bc = np.asarray(inputs["bc"], dtype=np.float32)
    B = S * NCORES
    out = np.empty((B, C, H, W), dtype=np.float32)
    for i in range(NCORES):
        out[S * i : S * (i + 1)] = (
            outs[i]["out"].astype(np.float32).reshape(S, C, H, W)
        )
    out += bc[None, :, None, None]
    return out, res


def kernel(**inputs) -> np.ndarray:
    out, _ = run_on_hw(inputs)
    return out


if __name__ == "__main__":
    nc = build_program()
    print("program built OK")


# revision 53
# speedup vs baseline: 1.0256x; 1.0256x over previous
"""Trainium2 Bass kernel for nn_DA_conv: per-sample dynamic depthwise 3x3 conv
(+LeakyReLU) followed by a 1x1 pointwise conv, with the 3x3 kernels produced by
a small per-sample MLP.

Strategy (8 NeuronCores, pure batch data-parallel, 2 samples per core).

CoreSim cost model facts this design is built around (measured):
  - matmul charges output-free-size x 0.417ns regardless of partition count
    or tile_position, so every matmul uses all 128 partitions (2 samples x 64
    channels) via diagonal / block-diagonal weights.
  - DVE tensor_scalar runs at 4x for 2-byte SBUF operands (194ns/512),
    tensor_tensor at 2x (327ns/512), scalar_tensor_tensor gets no speedup.
  - Pool runs TS/TT/STT flat at 0.833ns/elem with no per-op init and may
    read-modify-write PSUM (853ns/1024).
  - Act activation costs ~(N*0.833 + 143..370)ns; Prelu alpha=1.0 is an
    identity copy on the same table (no act-table reload).
  - DMA is one serialized resource; contiguous chunks >=512B get 360GB/s,
    smaller chunks pay 2x.

So the 9 depthwise taps + the rest are split across engines (per-image
totals): PE 5 taps + 1x1 (41us), Pool 2 taps + partial->PSUM merge (41us),
DVE 2 taps + pair-merge + half output evac (30us), Act lrelu + half output
evac (26us), DMA fp16-in/fp16-out (23us).

The feature map lives in SBUF fp16 with rows contiguous (stride 128 = DRAM
layout, full-rate DMA) plus one zero row above/below. Horizontal SAME-padding
is realized by TRIMMING the access patterns of the dj!=1 taps (the border
column simply doesn't receive that tap) instead of padding columns, which
keeps every DMA contiguous.
"""

import os
import sys

sys.path.insert(0, "/opt/trn_rl_repo")

from contextlib import ExitStack

import numpy as np

import concourse.bacc as bacc
import concourse.bass as bass
import concourse.mybir as mybir
import concourse.tile as tile

S = 2            # samples per core
C = 64           # channels
H = W = 128      # spatial
KK = 3           # conv kernel size
NCORES = 8
RP = H + 2       # stored rows (zero halo row above and below)
SR = 8           # image rows per super-tile
NST = H // SR    # 16 super-tiles
SPX = SR * W     # 1024 pixels per super-tile (2 PSUM banks)

f32 = mybir.dt.float32
f16 = mybir.dt.float16
i32 = mybir.dt.int32

X_MODE = "f16"   # kept for test.py compatibility

LRELU = mybir.ActivationFunctionType.Prelu

# tap split across engines: (di, dj) with di=vertical, dj=horizontal.
# GPSIMD/Pool supports no generic elementwise ops on HW, so PE and DVE carry
# everything, balanced at ~49us each. DVE takes 4 taps: the two full-width
# center-column taps plus the (0,0)/(2,0) pair -- both dj=0, so their
# x-aligned full-width tensor_scalar products (4x DVE mode) merge with one
# full tensor_tensor and one column-trimmed tensor_tensor. PE carries the
# other 5 taps as diagonal matmuls, adds v into PSUM with an identity matmul
# inside the accumulation group, and runs the 1x1.
PE_TAPS = [(1, 1), (0, 2), (2, 2), (1, 0), (1, 2)]   # (1,1) first: start=True
DVE_TAPS = [(0, 1), (2, 1)]                          # full-width center column
DVE_PAIR = [(0, 0), (2, 0)]                          # dj=0 pair, trimmed merge


def _cols(dj):
    """(in_col_slice, out_col_slice) realizing horizontal SAME padding."""
    if dj == 0:
        return slice(0, W - 1), slice(1, W)
    if dj == 1:
        return slice(0, W), slice(0, W)
    return slice(1, W), slice(0, W - 1)


def build_program() -> bass.Bass:
    nc = bacc.Bacc("TRN2", target_bir_lowering=False, debug=False)

    x_d = nc.dram_tensor("x", [S * C, H * W], f16, kind="ExternalInput").ap()
    # d2[s*64+i] = d[s, i] (both samples stacked in one column)
    d2_d = nc.dram_tensor("d2", [S * C, 1], f32, kind="ExternalInput").ap()
    # block-diag(Wk1.T, Wk1.T)
    wk1_d = nc.dram_tensor("wk1b", [S * C, S * C], f32,
                           kind="ExternalInput").ap()
    # per-tap block-diag(Wk2_t.T, Wk2_t.T): wk2b[j2, t*128 + p]
    wk2_d = nc.dram_tensor("wk2b", [S * C, KK * KK * S * C], f16,
                           kind="ExternalInput").ap()
    # block-diag(Wc.T, Wc.T) in fp16
    wc2_d = nc.dram_tensor("wc2", [S * C, S * C], f16, kind="ExternalInput").ap()
    out_d = nc.dram_tensor("out", [S * C, H * W], f16, kind="ExternalOutput").ap()

    with tile.TileContext(nc) as tc, ExitStack() as ctx:
        _body(ctx, tc, x_d, d2_d, wk1_d, wk2_d, wc2_d, out_d)
    nc.compile()
    return nc


def _body(ctx, tc, x_d, d2_d, wk1_d, wk2_d, wc2_d, out_d):
    nc = tc.nc
    P128 = S * C
    const = ctx.enter_context(tc.tile_pool(name="const", bufs=1))
    xpool = ctx.enter_context(tc.tile_pool(name="xs", bufs=1))
    vpool = ctx.enter_context(tc.tile_pool(name="v", bufs=3))
    apool = ctx.enter_context(tc.tile_pool(name="a", bufs=3))
    zpool = ctx.enter_context(tc.tile_pool(name="z", bufs=2))
    pdw = ctx.enter_context(tc.tile_pool(name="pdw", bufs=2, space="PSUM"))
    po = ctx.enter_context(tc.tile_pool(name="po", bufs=4, space="PSUM"))

    # Preload the Prelu activation table while the weight DMAs are in
    # flight; otherwise the 1283ns table load lands on the MLP critical path.
    warm = const.tile([C, 1], f32)
    nc.vector.memset(warm[:, :], 0.0)
    nc.scalar.activation(warm[:, :], warm[:, :], LRELU, alpha=0.1)

    # ---------------- small-weight loads (MLP deps first) ----------------
    wk1b = const.tile([P128, P128], f32)
    nc.sync.dma_start(wk1b[:, :], wk1_d)
    d2s = const.tile([P128, 1], f32)
    nc.sync.dma_start(d2s[:, :], d2_d)
    # ---------------- resident feature map (fp16, contiguous rows) ----------
    # first two x chunks jump the DMA queue ahead of the big weights so the
    # DVE tap products can start as early as possible
    xs = xpool.tile([P128, RP * W], f16)
    nc.vector.memset(xs[:, 0:W], 0.0)
    nc.vector.memset(xs[:, (RP - 1) * W : RP * W], 0.0)

    def xchunk(k):
        dst = xs[:, (1 + k * SR) * W : (1 + (k + 1) * SR) * W]
        nc.sync.dma_start(dst, x_d[:, k * SPX : (k + 1) * SPX])

    wk2b = const.tile([P128, KK * KK * P128], f16)
    nc.sync.dma_start(wk2b[:, :], wk2_d)
    for k in range(2):
        xchunk(k)
    wc2 = const.tile([P128, P128], f16)
    nc.sync.dma_start(wc2[:, :], wc2_d)
    for k in range(2, NST):
        xchunk(k)

    # ---------------- kernel-generating MLP ----------------
    hid_ps = po.tile([P128, 1], f32, tag="o")
    nc.tensor.matmul(hid_ps[:, :], lhsT=wk1b[:, :], rhs=d2s[:, :],
                     start=True, stop=True)
    # lrelu(x) = max(x, 0.1x) on DVE -- keeps the MLP critical path off the
    # Activation engine (whose queue holds the act-table preload).
    hid_t = const.tile([P128, 1], f32)
    nc.vector.tensor_scalar_mul(hid_t[:, :], hid_ps[:, :], 0.1)
    hid_sb = const.tile([P128, 1], f16)
    nc.vector.tensor_tensor(hid_sb[:, :], hid_ps[:, :], hid_t[:, :],
                            op=mybir.AluOpType.max)

    # identity matrix (independent of the MLP; emitted early)
    id_i = const.tile([P128, P128], i32)
    nc.gpsimd.iota(id_i[:, :], pattern=[[1, P128]], base=0,
                   channel_multiplier=-1)
    idf = const.tile([P128, P128], f16)
    nc.vector.tensor_scalar(idf[:, :], id_i[:, :], 0, None,
                            mybir.AluOpType.is_equal)

    # kcols[s*64+c, t] = kern[s, c*9+t]  (fp32, per-partition tap scalars) and
    # fp16 diagonal weight matrices, built in PE-tap order so the first
    # depthwise matmul can issue as early as possible.
    # each tap's block-diag Wk2 matmul produces kern for all 128 (s,c) pairs
    # as one [128,1] column; the PSUM->SBUF copies run on the (idle) Act
    # engine so DVE can start tap products immediately.
    ntap = KK * KK
    kcols = const.tile([P128, ntap], f32)
    diag = const.tile([P128, ntap * P128], f16)
    tap_order = [PE_TAPS[0]] + DVE_TAPS + DVE_PAIR + PE_TAPS[1:]
    for (di, dj) in tap_order:
        t = di * KK + dj
        kp = po.tile([P128, 1], f32, tag="o")
        nc.tensor.matmul(kp[:, :], lhsT=wk2b[:, t * P128 : (t + 1) * P128],
                         rhs=hid_sb[:, :], start=True, stop=True)
        if (di, dj) in PE_TAPS:
            nc.scalar.activation(kcols[:, t : t + 1], kp[:, :],
                                 mybir.ActivationFunctionType.Copy)
            nc.vector.tensor_scalar_mul(diag[:, t * P128 : (t + 1) * P128],
                                        idf[:, :], kcols[:, t : t + 1])
        else:
            nc.vector.tensor_copy(kcols[:, t : t + 1], kp[:, :])

    # ---------------- main loop ----------------
    xr = xs[:, :].rearrange("p (r w) -> p r w", w=W)

    def kap(di, dj):
        return kcols[:, (di * KK + dj) : (di * KK + dj) + 1]

    def dve_products(k0, nt):
        # tap products for nt consecutive super-tiles in one go -- larger TS
        # ops amortize the per-op DVE init (4x DVE mode). All four products
        # are full-width x-aligned.
        parts = []
        for i, (di, dj) in enumerate(DVE_TAPS + DVE_PAIR):
            t = vpool.tile([P128, nt * SPX], f16, tag=f"t{i}")
            w = xr[:, SR * k0 + di : SR * k0 + di + nt * SR, :]
            nc.vector.tensor_scalar_mul(t[:, :], w, kap(di, dj))
            parts.append(t)
        return parts

    def dve_merge(k, parts, off):
        # per-tile TT merges keep v latency tile-local; the dj=0 pair merges
        # via a column-trimmed TT, realizing SAME padding (col 0 gets no
        # dj=0 contribution).
        t0, t1, t2, t3 = parts
        sl = slice(off, off + SPX)
        v = vpool.tile([P128, SPX], f16, tag="v")
        nc.vector.tensor_tensor(v[:, :], t0[:, sl], t1[:, sl],
                                op=mybir.AluOpType.add)
        nc.vector.tensor_tensor(t2[:, sl], t2[:, sl], t3[:, sl],
                                op=mybir.AluOpType.add)
        vv = v[:, :].rearrange("p (r w) -> p r w", w=W)
        qv = t2[:, sl].rearrange("p (r w) -> p r w", w=W)
        nc.vector.tensor_tensor(vv[:, :, 1:W], vv[:, :, 1:W],
                                qv[:, :, 0 : W - 1], op=mybir.AluOpType.add)
        return v

    def pe_taps_half(k, P, h):
        Pv = P[:, :].rearrange("p (r w) -> p r w", w=W)
        out = Pv[:, 4 * h : 4 * h + 4, :]
        r0 = SR * k + 4 * h
        for i, (di, dj) in enumerate(PE_TAPS):
            ci, co = _cols(dj)
            nc.tensor.matmul(
                out[:, :, co],
                lhsT=diag[:, (di * KK + dj) * P128 : (di * KK + dj + 1) * P128],
                rhs=xr[:, r0 + di : r0 + di + 4, ci],
                start=(i == 0), stop=False,
                skip_group_check=True)

    def pe_iadd_half(k, P, v, h):
        # identity matmul folds the DVE partial into the accumulation group
        # (stop=True): no cross-engine PSUM merge needed.
        nc.tensor.matmul(
            P[:, 512 * h : 512 * (h + 1)], lhsT=idf[:, :],
            rhs=v[:, 512 * h : 512 * (h + 1)],
            start=False, stop=True, skip_group_check=True)

    def pe_taps(k):
        P = pdw.tile([P128, SPX], f32, tag="p")
        for h in (0, 1):
            pe_taps_half(k, P, h)
        return P

    def pe_iadd(k, P, v):
        for h in (0, 1):
            pe_iadd_half(k, P, v, h)

    def prelu_stage(k, P):
        # Full-width in steady state; half granularity in the last tiles
        # keeps the pipeline drain short.
        halves = k >= NST - 2
        a = apool.tile([P128, SPX], f16, tag="a")
        for hs in ([slice(0, 512), slice(512, SPX)] if halves
                   else [slice(0, SPX)]):
            nc.scalar.activation(a[:, hs], P[:, hs], LRELU, alpha=0.1)
        return a

    def conv1x1_half(k, h, a):
        O = po.tile([P128, 512], f32, tag="o")
        nc.tensor.matmul(O[:, :], lhsT=wc2[:, :], rhs=a[:, 512 * h : 512 * (h + 1)],
                         start=True, stop=True)
        z = zpool.tile([P128, 512], f16, tag=f"z{h}")
        if k >= NST - 1:
            # Act is congested with prelu halves during the drain; DVE idles
            nc.vector.tensor_copy(z[:, :], O[:, :])
        else:
            nc.scalar.activation(z[:, :], O[:, :], LRELU, alpha=1.0)
        nc.sync.dma_start(out_d[:, k * SPX + 512 * h : k * SPX + 512 * (h + 1)],
                          z[:, :])

    # DVE product granularity: single tiles at the pipeline ends (short
    # v latency for fill/drain), 4-tile blocks in the middle (lower DVE op
    # overhead)
    blocks = [(0, 1), (1, 1)] + [(k, 2) for k in range(2, NST - 2, 2)] + [
        (NST - 2, 1), (NST - 1, 1)]
    tiles = []
    for (k0, nt) in blocks:
        tiles.extend((k0, nt, i) for i in range(nt))

    pending = []
    parts = None
    for (k0, nt, i) in tiles:
        k = k0 + i
        if i == 0:
            parts = dve_products(k0, nt)
        v = dve_merge(k, parts, SPX * i)
        if k == NST - 1:
            # last tile: interleave per half so the drain chain starts at
            # the tile's midpoint
            P = pdw.tile([P128, SPX], f32, tag="p")
            a = apool.tile([P128, SPX], f16, tag="a")
            for h in (0, 1):
                pe_taps_half(k, P, h)
                pe_iadd_half(k, P, v, h)
                nc.scalar.activation(a[:, 512 * h : 512 * (h + 1)],
                                     P[:, 512 * h : 512 * (h + 1)],
                                     LRELU, alpha=0.1)
        else:
            P = pe_taps(k)
            pe_iadd(k, P, v)
            a = prelu_stage(k, P)
        for item in pending:
            conv1x1_half(*item)
        pending = [(k, 0, a), (k, 1, a)]
    for item in pending:
        conv1x1_half(*item)


# ---------------------------------------------------------------------------
# host-side entry point
# ---------------------------------------------------------------------------

_PROGRAM_CACHE: dict[str, bass.Bass] = {}


def _get_program(x_mode: str = X_MODE) -> bass.Bass:
    if x_mode not in _PROGRAM_CACHE:
        _PROGRAM_CACHE[x_mode] = build_program()
    return _PROGRAM_CACHE[x_mode]


def _host_prep(inputs: dict, x_mode: str = X_MODE):
    x = np.asarray(inputs["x"], dtype=np.float32)
    d = np.asarray(inputs["d"], dtype=np.float32)
    Wk1 = np.asarray(inputs["Wk1"], dtype=np.float32)
    Wk2 = np.asarray(inputs["Wk2"], dtype=np.float32)
    Wc = np.asarray(inputs["Wc"], dtype=np.float32)

    wk1b = np.zeros((S * C, S * C), dtype=np.float32)
    wk1b[0:C, 0:C] = Wk1.T
    wk1b[C:, C:] = Wk1.T

    # wk2b[:, t*128:(t+1)*128] = block-diag(Wk2_t.T, Wk2_t.T),
    # where Wk2_t[c, j] = Wk2[c*9+t, j]
    w = Wk2.reshape(C, KK * KK, C)                    # (c, t, j)
    wk2b = np.zeros((S * C, KK * KK, S * C), dtype=np.float16)
    wk2b[0:C, :, 0:C] = w.transpose(2, 1, 0)          # (j, t, c)
    wk2b[C:, :, C:] = w.transpose(2, 1, 0)
    wk2b = np.ascontiguousarray(
        wk2b.transpose(0, 1, 2).reshape(S * C, KK * KK * S * C)
    )

    wc2 = np.zeros((S * C, S * C), dtype=np.float16)
    wc2[0:C, 0:C] = Wc.T
    wc2[C:, C:] = Wc.T

    xcast = x.astype(np.float16)

    in_maps = []
    for i in range(NCORES):
        xi = np.ascontiguousarray(xcast[S * i : S * (i + 1)].reshape(S * C, H * W))
        d2 = np.ascontiguousarray(
            d[S * i : S * (i + 1)].reshape(S * C, 1)
        )
        in_maps.append(
            {"x": xi, "d2": d2, "wk1b": wk1b, "wk2b": wk2b, "wc2": wc2}
        )
    return in_maps


def run_on_hw(inputs: dict, x_mode: str = X_MODE, **kwargs):
    """Run the SPMD kernel on 8 NeuronCores; returns (output, results)."""
    from concourse.bass_utils import run_bass_kernel_spmd

    nc = _get_program(x_mode)
    in_maps = _host_prep(inputs, x_mode)
    res = run_bass_kernel_spmd(nc, in_maps, core_ids=list(range(NCORES)), **kwargs)
    outs = res.results
    bc = np.asarray(inputs["bc"], dtype=np.float32)
    B = S * NCORES
    out = np.empty((B, C, H, W), dtype=np.float32)
    for i in range(NCORES):
        out[S * i : S * (i + 1)] = (
            outs[i]["out"].astype(np.float32).reshape(S, C, H, W)
        )
    out += bc[None, :, None, None]
    return out, res


def kernel(**inputs) -> np.ndarray:
    out, _ = run_on_hw(inputs)
    return out


if __name__ == "__main__":
    nc = build_program()
    print("program built OK")


# revision 54
# speedup vs baseline: 1.0362x; 1.0104x over previous
"""Trainium2 Bass kernel for nn_DA_conv: per-sample dynamic depthwise 3x3 conv
(+LeakyReLU) followed by a 1x1 pointwise conv, with the 3x3 kernels produced by
a small per-sample MLP.

Strategy (8 NeuronCores, pure batch data-parallel, 2 samples per core).

CoreSim cost model facts this design is built around (measured):
  - matmul charges output-free-size x 0.417ns regardless of partition count
    or tile_position, so every matmul uses all 128 partitions (2 samples x 64
    channels) via diagonal / block-diagonal weights.
  - DVE tensor_scalar runs at 4x for 2-byte SBUF operands (194ns/512),
    tensor_tensor at 2x (327ns/512), scalar_tensor_tensor gets no speedup.
  - Pool runs TS/TT/STT flat at 0.833ns/elem with no per-op init and may
    read-modify-write PSUM (853ns/1024).
  - Act activation costs ~(N*0.833 + 143..370)ns; Prelu alpha=1.0 is an
    identity copy on the same table (no act-table reload).
  - DMA is one serialized resource; contiguous chunks >=512B get 360GB/s,
    smaller chunks pay 2x.

So the 9 depthwise taps + the rest are split across engines (per-image
totals): PE 5 taps + 1x1 (41us), Pool 2 taps + partial->PSUM merge (41us),
DVE 2 taps + pair-merge + half output evac (30us), Act lrelu + half output
evac (26us), DMA fp16-in/fp16-out (23us).

The feature map lives in SBUF fp16 with rows contiguous (stride 128 = DRAM
layout, full-rate DMA) plus one zero row above/below. Horizontal SAME-padding
is realized by TRIMMING the access patterns of the dj!=1 taps (the border
column simply doesn't receive that tap) instead of padding columns, which
keeps every DMA contiguous.
"""

import os
import sys

sys.path.insert(0, "/opt/trn_rl_repo")

from contextlib import ExitStack

import numpy as np

import concourse.bacc as bacc
import concourse.bass as bass
import concourse.mybir as mybir
import concourse.tile as tile

S = 2            # samples per core
C = 64           # channels
H = W = 128      # spatial
KK = 3           # conv kernel size
NCORES = 8
RP = H + 2       # stored rows (zero halo row above and below)
SR = 8           # image rows per super-tile
NST = H // SR    # 16 super-tiles
SPX = SR * W     # 1024 pixels per super-tile (2 PSUM banks)

f32 = mybir.dt.float32
f16 = mybir.dt.float16
i32 = mybir.dt.int32

X_MODE = "f16"   # kept for test.py compatibility

LRELU = mybir.ActivationFunctionType.Prelu

# tap split across engines: (di, dj) with di=vertical, dj=horizontal.
# GPSIMD/Pool supports no generic elementwise ops on HW, so PE and DVE carry
# everything, balanced at ~49us each. DVE takes 4 taps: the two full-width
# center-column taps plus the (0,0)/(2,0) pair -- both dj=0, so their
# x-aligned full-width tensor_scalar products (4x DVE mode) merge with one
# full tensor_tensor and one column-trimmed tensor_tensor. PE carries the
# other 5 taps as diagonal matmuls, adds v into PSUM with an identity matmul
# inside the accumulation group, and runs the 1x1.
PE_TAPS = [(1, 1), (0, 2), (2, 2), (1, 0), (1, 2)]   # (1,1) first: start=True
DVE_TAPS = [(0, 1), (2, 1)]                          # full-width center column
DVE_PAIR = [(0, 0), (2, 0)]                          # dj=0 pair, trimmed merge


def _cols(dj):
    """(in_col_slice, out_col_slice) realizing horizontal SAME padding."""
    if dj == 0:
        return slice(0, W - 1), slice(1, W)
    if dj == 1:
        return slice(0, W), slice(0, W)
    return slice(1, W), slice(0, W - 1)


def build_program() -> bass.Bass:
    nc = bacc.Bacc("TRN2", target_bir_lowering=False, debug=False)

    x_d = nc.dram_tensor("x", [S * C, H * W], f16, kind="ExternalInput").ap()
    # d2[s*64+i] = d[s, i] (both samples stacked in one column)
    d2_d = nc.dram_tensor("d2", [S * C, 1], f32, kind="ExternalInput").ap()
    # block-diag(Wk1.T, Wk1.T)
    wk1_d = nc.dram_tensor("wk1b", [S * C, S * C], f32,
                           kind="ExternalInput").ap()
    # per-tap block-diag(Wk2_t.T, Wk2_t.T): wk2b[j2, t*128 + p]
    wk2_d = nc.dram_tensor("wk2b", [S * C, KK * KK * S * C], f16,
                           kind="ExternalInput").ap()
    # block-diag(Wc.T, Wc.T) in fp16
    wc2_d = nc.dram_tensor("wc2", [S * C, S * C], f16, kind="ExternalInput").ap()
    out_d = nc.dram_tensor("out", [S * C, H * W], f16, kind="ExternalOutput").ap()

    with tile.TileContext(nc) as tc, ExitStack() as ctx:
        _body(ctx, tc, x_d, d2_d, wk1_d, wk2_d, wc2_d, out_d)
    nc.compile()
    return nc


def _body(ctx, tc, x_d, d2_d, wk1_d, wk2_d, wc2_d, out_d):
    nc = tc.nc
    P128 = S * C
    const = ctx.enter_context(tc.tile_pool(name="const", bufs=1))
    xpool = ctx.enter_context(tc.tile_pool(name="xs", bufs=1))
    vpool = ctx.enter_context(tc.tile_pool(name="v", bufs=3))
    apool = ctx.enter_context(tc.tile_pool(name="a", bufs=3))
    zpool = ctx.enter_context(tc.tile_pool(name="z", bufs=2))
    pdw = ctx.enter_context(tc.tile_pool(name="pdw", bufs=2, space="PSUM"))
    po = ctx.enter_context(tc.tile_pool(name="po", bufs=4, space="PSUM"))

    # Preload the Prelu activation table while the weight DMAs are in
    # flight; otherwise the 1283ns table load lands on the MLP critical path.
    warm = const.tile([C, 1], f32)
    nc.vector.memset(warm[:, :], 0.0)
    nc.scalar.activation(warm[:, :], warm[:, :], LRELU, alpha=0.1)

    # ---------------- small-weight loads (MLP deps first) ----------------
    wk1b = const.tile([P128, P128], f32)
    nc.sync.dma_start(wk1b[:, :], wk1_d)
    d2s = const.tile([P128, 1], f32)
    nc.sync.dma_start(d2s[:, :], d2_d)
    # ---------------- resident feature map (fp16, contiguous rows) ----------
    # first two x chunks jump the DMA queue ahead of the big weights so the
    # DVE tap products can start as early as possible
    xs = xpool.tile([P128, RP * W], f16)
    nc.vector.memset(xs[:, 0:W], 0.0)
    nc.vector.memset(xs[:, (RP - 1) * W : RP * W], 0.0)

    def xchunk(k):
        dst = xs[:, (1 + k * SR) * W : (1 + (k + 1) * SR) * W]
        nc.sync.dma_start(dst, x_d[:, k * SPX : (k + 1) * SPX])

    wk2b = const.tile([P128, KK * KK * P128], f16)
    nc.sync.dma_start(wk2b[:, :], wk2_d)
    for k in range(2):
        xchunk(k)
    wc2 = const.tile([P128, P128], f16)
    nc.sync.dma_start(wc2[:, :], wc2_d)
    for k in range(2, NST):
        xchunk(k)

    # ---------------- kernel-generating MLP ----------------
    hid_ps = po.tile([P128, 1], f32, tag="o")
    nc.tensor.matmul(hid_ps[:, :], lhsT=wk1b[:, :], rhs=d2s[:, :],
                     start=True, stop=True)
    # lrelu(x) = max(x, 0.1x) on DVE -- keeps the MLP critical path off the
    # Activation engine (whose queue holds the act-table preload).
    hid_t = const.tile([P128, 1], f32)
    nc.vector.tensor_scalar_mul(hid_t[:, :], hid_ps[:, :], 0.1)
    hid_sb = const.tile([P128, 1], f16)
    nc.vector.tensor_tensor(hid_sb[:, :], hid_ps[:, :], hid_t[:, :],
                            op=mybir.AluOpType.max)

    # identity matrix (independent of the MLP; emitted early)
    id_i = const.tile([P128, P128], i32)
    nc.gpsimd.iota(id_i[:, :], pattern=[[1, P128]], base=0,
                   channel_multiplier=-1)
    idf = const.tile([P128, P128], f16)
    nc.vector.tensor_scalar(idf[:, :], id_i[:, :], 0, None,
                            mybir.AluOpType.is_equal)

    # kcols[s*64+c, t] = kern[s, c*9+t]  (fp32, per-partition tap scalars) and
    # fp16 diagonal weight matrices, built in PE-tap order so the first
    # depthwise matmul can issue as early as possible.
    # each tap's block-diag Wk2 matmul produces kern for all 128 (s,c) pairs
    # as one [128,1] column; the PSUM->SBUF copies run on the (idle) Act
    # engine so DVE can start tap products immediately.
    ntap = KK * KK
    kcols = const.tile([P128, ntap], f32)
    diag = const.tile([P128, ntap * P128], f16)
    tap_order = [PE_TAPS[0]] + DVE_TAPS + DVE_PAIR + PE_TAPS[1:]
    for (di, dj) in tap_order:
        t = di * KK + dj
        kp = po.tile([P128, 1], f32, tag="o")
        nc.tensor.matmul(kp[:, :], lhsT=wk2b[:, t * P128 : (t + 1) * P128],
                         rhs=hid_sb[:, :], start=True, stop=True)
        if (di, dj) in PE_TAPS:
            nc.scalar.activation(kcols[:, t : t + 1], kp[:, :],
                                 mybir.ActivationFunctionType.Copy)
            nc.vector.tensor_scalar_mul(diag[:, t * P128 : (t + 1) * P128],
                                        idf[:, :], kcols[:, t : t + 1])
        else:
            nc.vector.tensor_copy(kcols[:, t : t + 1], kp[:, :])

    # ---------------- main loop ----------------
    xr = xs[:, :].rearrange("p (r w) -> p r w", w=W)

    def kap(di, dj):
        return kcols[:, (di * KK + dj) : (di * KK + dj) + 1]

    def dve_products(k0, nt):
        # tap products for nt consecutive super-tiles in one go -- larger TS
        # ops amortize the per-op DVE init (4x DVE mode). All four products
        # are full-width x-aligned.
        parts = []
        for i, (di, dj) in enumerate(DVE_TAPS + DVE_PAIR):
            t = vpool.tile([P128, nt * SPX], f16, tag=f"t{i}")
            w = xr[:, SR * k0 + di : SR * k0 + di + nt * SR, :]
            nc.vector.tensor_scalar_mul(t[:, :], w, kap(di, dj))
            parts.append(t)
        return parts

    def dve_merge(k, parts, off):
        # per-tile TT merges keep v latency tile-local; the dj=0 pair merges
        # via a column-trimmed TT, realizing SAME padding (col 0 gets no
        # dj=0 contribution).
        t0, t1, t2, t3 = parts
        sl = slice(off, off + SPX)
        v = vpool.tile([P128, SPX], f16, tag="v")
        nc.vector.tensor_tensor(v[:, :], t0[:, sl], t1[:, sl],
                                op=mybir.AluOpType.add)
        nc.vector.tensor_tensor(t2[:, sl], t2[:, sl], t3[:, sl],
                                op=mybir.AluOpType.add)
        vv = v[:, :].rearrange("p (r w) -> p r w", w=W)
        qv = t2[:, sl].rearrange("p (r w) -> p r w", w=W)
        nc.vector.tensor_tensor(vv[:, :, 1:W], vv[:, :, 1:W],
                                qv[:, :, 0 : W - 1], op=mybir.AluOpType.add)
        return v

    def pe_taps_half(k, P, h):
        Pv = P[:, :].rearrange("p (r w) -> p r w", w=W)
        out = Pv[:, 4 * h : 4 * h + 4, :]
        r0 = SR * k + 4 * h
        for i, (di, dj) in enumerate(PE_TAPS):
            ci, co = _cols(dj)
            nc.tensor.matmul(
                out[:, :, co],
                lhsT=diag[:, (di * KK + dj) * P128 : (di * KK + dj + 1) * P128],
                rhs=xr[:, r0 + di : r0 + di + 4, ci],
                start=(i == 0), stop=False,
                skip_group_check=True)

    def pe_iadd_half(k, P, v, h):
        # identity matmul folds the DVE partial into the accumulation group
        # (stop=True): no cross-engine PSUM merge needed.
        nc.tensor.matmul(
            P[:, 512 * h : 512 * (h + 1)], lhsT=idf[:, :],
            rhs=v[:, 512 * h : 512 * (h + 1)],
            start=False, stop=True, skip_group_check=True)

    def pe_taps(k):
        P = pdw.tile([P128, SPX], f32, tag="p")
        for h in (0, 1):
            pe_taps_half(k, P, h)
        return P

    def pe_iadd(k, P, v):
        for h in (0, 1):
            pe_iadd_half(k, P, v, h)

    def prelu_stage(k, P):
        # Full-width in steady state; half granularity in the last tiles
        # keeps the pipeline drain short.
        halves = k >= NST - 2
        a = apool.tile([P128, SPX], f16, tag="a")
        for hs in ([slice(0, 512), slice(512, SPX)] if halves
                   else [slice(0, SPX)]):
            nc.scalar.activation(a[:, hs], P[:, hs], LRELU, alpha=0.1)
        return a

    def conv1x1_half(k, h, a):
        O = po.tile([P128, 512], f32, tag="o")
        nc.tensor.matmul(O[:, :], lhsT=wc2[:, :], rhs=a[:, 512 * h : 512 * (h + 1)],
                         start=True, stop=True)
        z = zpool.tile([P128, 512], f16, tag=f"z{h}")
        if k >= NST - 2:
            # Act is congested with prelu halves during the drain; DVE idles
            nc.vector.tensor_copy(z[:, :], O[:, :])
        else:
            nc.scalar.activation(z[:, :], O[:, :], LRELU, alpha=1.0)
        nc.sync.dma_start(out_d[:, k * SPX + 512 * h : k * SPX + 512 * (h + 1)],
                          z[:, :])

    # DVE product granularity: single tiles at the pipeline ends (short
    # v latency for fill/drain), 4-tile blocks in the middle (lower DVE op
    # overhead)
    blocks = [(0, 1), (1, 1)] + [(k, 2) for k in range(2, NST - 2, 2)] + [
        (NST - 2, 1), (NST - 1, 1)]
    tiles = []
    for (k0, nt) in blocks:
        tiles.extend((k0, nt, i) for i in range(nt))

    pending = []
    parts = None
    for (k0, nt, i) in tiles:
        k = k0 + i
        if i == 0:
            parts = dve_products(k0, nt)
        v = dve_merge(k, parts, SPX * i)
        if k == NST - 1:
            # last tile: interleave per half so the drain chain starts at
            # the tile's midpoint
            P = pdw.tile([P128, SPX], f32, tag="p")
            a = apool.tile([P128, SPX], f16, tag="a")
            for h in (0, 1):
                pe_taps_half(k, P, h)
                pe_iadd_half(k, P, v, h)
                nc.scalar.activation(a[:, 512 * h : 512 * (h + 1)],
                                     P[:, 512 * h : 512 * (h + 1)],
                                     LRELU, alpha=0.1)
        else:
            P = pe_taps(k)
            pe_iadd(k, P, v)
            a = prelu_stage(k, P)
        for item in pending:
            conv1x1_half(*item)
        pending = [(k, 0, a), (k, 1, a)]
    for item in pending:
        conv1x1_half(*item)


# ---------------------------------------------------------------------------
# host-side entry point
# ---------------------------------------------------------------------------

_PROGRAM_CACHE: dict[str, bass.Bass] = {}


def _get_program(x_mode: str = X_MODE) -> bass.Bass:
    if x_mode not in _PROGRAM_CACHE:
        _PROGRAM_CACHE[x_mode] = build_program()
    return _PROGRAM_CACHE[x_mode]


def _host_prep(inputs: dict, x_mode: str = X_MODE):
    x = np.asarray(inputs["x"], dtype=np.float32)
    d = np.asarray(inputs["d"], dtype=np.float32)
    Wk1 = np.asarray(inputs["Wk1"], dtype=np.float32)
    Wk2 = np.asarray(inputs["Wk2"], dtype=np.float32)
    Wc = np.asarray(inputs["Wc"], dtype=np.float32)

    wk1b = np.zeros((S * C, S * C), dtype=np.float32)
    wk1b[0:C, 0:C] = Wk1.T
    wk1b[C:, C:] = Wk1.T

    # wk2b[:, t*128:(t+1)*128] = block-diag(Wk2_t.T, Wk2_t.T),
    # where Wk2_t[c, j] = Wk2[c*9+t, j]
    w = Wk2.reshape(C, KK * KK, C)                    # (c, t, j)
    wk2b = np.zeros((S * C, KK * KK, S * C), dtype=np.float16)
    wk2b[0:C, :, 0:C] = w.transpose(2, 1, 0)          # (j, t, c)
    wk2b[C:, :, C:] = w.transpose(2, 1, 0)
    wk2b = np.ascontiguousarray(
        wk2b.transpose(0, 1, 2).reshape(S * C, KK * KK * S * C)
    )

    wc2 = np.zeros((S * C, S * C), dtype=np.float16)
    wc2[0:C, 0:C] = Wc.T
    wc2[C:, C:] = Wc.T

    xcast = x.astype(np.float16)

    in_maps = []
    for i in range(NCORES):
        xi = np.ascontiguousarray(xcast[S * i : S * (i + 1)].reshape(S * C, H * W))
        d2 = np.ascontiguousarray(
            d[S * i : S * (i + 1)].reshape(S * C, 1)
        )
        in_maps.append(
            {"x": xi, "d2": d2, "wk1b": wk1b, "wk2b": wk2b, "wc2": wc2}
        )
    return in_maps


def run_on_hw(inputs: dict, x_mode: str = X_MODE, **kwargs):
    """Run the SPMD kernel on 8 NeuronCores; returns (output, results)."""
    from concourse.bass_utils import run_bass_kernel_spmd

    nc = _get_program(x_mode)
    in_maps = _host_prep(inputs, x_mode)
    res = run_bass_kernel_spmd(nc, in_maps, core_ids=list(range(NCORES)), **kwargs)
    outs = res.results
    bc = np.asarray(inputs["bc"], dtype=np.float32)
    B = S * NCORES
    out = np.empty((B, C, H, W), dtype=np.float32)
    for i in range(NCORES):
        out[S * i : S * (i + 1)] = (
            outs[i]["out"].astype(np.float32).reshape(S, C, H, W)
        )
    out += bc[None, :, None, None]
    return out, res


def kernel(**inputs) -> np.ndarray:
    out, _ = run_on_hw(inputs)
    return out


if __name__ == "__main__":
    nc = build_program()
    print("program built OK")


# revision 55
# speedup vs baseline: 1.0433x; 1.0068x over previous
"""Trainium2 Bass kernel for nn_DA_conv: per-sample dynamic depthwise 3x3 conv
(+LeakyReLU) followed by a 1x1 pointwise conv, with the 3x3 kernels produced by
a small per-sample MLP.

Strategy (8 NeuronCores, pure batch data-parallel, 2 samples per core).

CoreSim cost model facts this design is built around (measured):
  - matmul charges output-free-size x 0.417ns regardless of partition count
    or tile_position, so every matmul uses all 128 partitions (2 samples x 64
    channels) via diagonal / block-diagonal weights.
  - DVE tensor_scalar runs at 4x for 2-byte SBUF operands (194ns/512),
    tensor_tensor at 2x (327ns/512), scalar_tensor_tensor gets no speedup.
  - Pool runs TS/TT/STT flat at 0.833ns/elem with no per-op init and may
    read-modify-write PSUM (853ns/1024).
  - Act activation costs ~(N*0.833 + 143..370)ns; Prelu alpha=1.0 is an
    identity copy on the same table (no act-table reload).
  - DMA is one serialized resource; contiguous chunks >=512B get 360GB/s,
    smaller chunks pay 2x.

So the 9 depthwise taps + the rest are split across engines (per-image
totals): PE 5 taps + 1x1 (41us), Pool 2 taps + partial->PSUM merge (41us),
DVE 2 taps + pair-merge + half output evac (30us), Act lrelu + half output
evac (26us), DMA fp16-in/fp16-out (23us).

The feature map lives in SBUF fp16 with rows contiguous (stride 128 = DRAM
layout, full-rate DMA) plus one zero row above/below. Horizontal SAME-padding
is realized by TRIMMING the access patterns of the dj!=1 taps (the border
column simply doesn't receive that tap) instead of padding columns, which
keeps every DMA contiguous.
"""

import os
import sys

sys.path.insert(0, "/opt/trn_rl_repo")

from contextlib import ExitStack

import numpy as np

import concourse.bacc as bacc
import concourse.bass as bass
import concourse.mybir as mybir
import concourse.tile as tile

S = 2            # samples per core
C = 64           # channels
H = W = 128      # spatial
KK = 3           # conv kernel size
NCORES = 8
RP = H + 2       # stored rows (zero halo row above and below)
SR = 8           # image rows per super-tile
NST = H // SR    # 16 super-tiles
SPX = SR * W     # 1024 pixels per super-tile (2 PSUM banks)

f32 = mybir.dt.float32
f16 = mybir.dt.float16
i32 = mybir.dt.int32

X_MODE = "f16"   # kept for test.py compatibility

LRELU = mybir.ActivationFunctionType.Prelu

# tap split across engines: (di, dj) with di=vertical, dj=horizontal.
# GPSIMD/Pool supports no generic elementwise ops on HW, so PE and DVE carry
# everything, balanced at ~49us each. DVE takes 4 taps: the two full-width
# center-column taps plus the (0,0)/(2,0) pair -- both dj=0, so their
# x-aligned full-width tensor_scalar products (4x DVE mode) merge with one
# full tensor_tensor and one column-trimmed tensor_tensor. PE carries the
# other 5 taps as diagonal matmuls, adds v into PSUM with an identity matmul
# inside the accumulation group, and runs the 1x1.
PE_TAPS = [(1, 1), (0, 2), (2, 2), (1, 0), (1, 2)]   # (1,1) first: start=True
DVE_TAPS = [(0, 1), (2, 1)]                          # full-width center column
DVE_PAIR = [(0, 0), (2, 0)]                          # dj=0 pair, trimmed merge


def _cols(dj):
    """(in_col_slice, out_col_slice) realizing horizontal SAME padding."""
    if dj == 0:
        return slice(0, W - 1), slice(1, W)
    if dj == 1:
        return slice(0, W), slice(0, W)
    return slice(1, W), slice(0, W - 1)


def build_program() -> bass.Bass:
    nc = bacc.Bacc("TRN2", target_bir_lowering=False, debug=False)

    x_d = nc.dram_tensor("x", [S * C, H * W], f16, kind="ExternalInput").ap()
    # d2[s*64+i] = d[s, i] (both samples stacked in one column)
    d2_d = nc.dram_tensor("d2", [S * C, 1], f32, kind="ExternalInput").ap()
    # block-diag(Wk1.T, Wk1.T)
    wk1_d = nc.dram_tensor("wk1b", [S * C, S * C], f32,
                           kind="ExternalInput").ap()
    # per-tap block-diag(Wk2_t.T, Wk2_t.T): wk2b[j2, t*128 + p]
    wk2_d = nc.dram_tensor("wk2b", [S * C, KK * KK * S * C], f16,
                           kind="ExternalInput").ap()
    # block-diag(Wc.T, Wc.T) in fp16
    wc2_d = nc.dram_tensor("wc2", [S * C, S * C], f16, kind="ExternalInput").ap()
    out_d = nc.dram_tensor("out", [S * C, H * W], f16, kind="ExternalOutput").ap()

    with tile.TileContext(nc) as tc, ExitStack() as ctx:
        _body(ctx, tc, x_d, d2_d, wk1_d, wk2_d, wc2_d, out_d)
    nc.compile()
    return nc


def _body(ctx, tc, x_d, d2_d, wk1_d, wk2_d, wc2_d, out_d):
    nc = tc.nc
    P128 = S * C
    const = ctx.enter_context(tc.tile_pool(name="const", bufs=1))
    xpool = ctx.enter_context(tc.tile_pool(name="xs", bufs=1))
    vpool = ctx.enter_context(tc.tile_pool(name="v", bufs=3))
    apool = ctx.enter_context(tc.tile_pool(name="a", bufs=3))
    zpool = ctx.enter_context(tc.tile_pool(name="z", bufs=2))
    pdw = ctx.enter_context(tc.tile_pool(name="pdw", bufs=2, space="PSUM"))
    po = ctx.enter_context(tc.tile_pool(name="po", bufs=4, space="PSUM"))

    # Preload the Prelu activation table while the weight DMAs are in
    # flight; otherwise the 1283ns table load lands on the MLP critical path.
    warm = const.tile([C, 1], f32)
    nc.vector.memset(warm[:, :], 0.0)
    nc.scalar.activation(warm[:, :], warm[:, :], LRELU, alpha=0.1)

    # ---------------- small-weight loads (MLP deps first) ----------------
    wk1b = const.tile([P128, P128], f32)
    nc.sync.dma_start(wk1b[:, :], wk1_d)
    d2s = const.tile([P128, 1], f32)
    nc.sync.dma_start(d2s[:, :], d2_d)
    # ---------------- resident feature map (fp16, contiguous rows) ----------
    # first two x chunks jump the DMA queue ahead of the big weights so the
    # DVE tap products can start as early as possible
    xs = xpool.tile([P128, RP * W], f16)
    nc.vector.memset(xs[:, 0:W], 0.0)
    nc.vector.memset(xs[:, (RP - 1) * W : RP * W], 0.0)

    def xchunk(k):
        dst = xs[:, (1 + k * SR) * W : (1 + (k + 1) * SR) * W]
        nc.sync.dma_start(dst, x_d[:, k * SPX : (k + 1) * SPX])

    wk2b = const.tile([P128, KK * KK * P128], f16)
    nc.sync.dma_start(wk2b[:, :], wk2_d)
    for k in range(2):
        xchunk(k)
    wc2 = const.tile([P128, P128], f16)
    nc.sync.dma_start(wc2[:, :], wc2_d)
    for k in range(2, NST):
        xchunk(k)

    # ---------------- kernel-generating MLP ----------------
    hid_ps = po.tile([P128, 1], f32, tag="o")
    nc.tensor.matmul(hid_ps[:, :], lhsT=wk1b[:, :], rhs=d2s[:, :],
                     start=True, stop=True)
    # lrelu(x) = max(x, 0.1x) on DVE -- keeps the MLP critical path off the
    # Activation engine (whose queue holds the act-table preload).
    hid_t = const.tile([P128, 1], f32)
    nc.vector.tensor_scalar_mul(hid_t[:, :], hid_ps[:, :], 0.1)
    hid_sb = const.tile([P128, 1], f16)
    nc.vector.tensor_tensor(hid_sb[:, :], hid_ps[:, :], hid_t[:, :],
                            op=mybir.AluOpType.max)

    # identity matrix (independent of the MLP; emitted early)
    id_i = const.tile([P128, P128], i32)
    nc.gpsimd.iota(id_i[:, :], pattern=[[1, P128]], base=0,
                   channel_multiplier=-1)
    idf = const.tile([P128, P128], f16)
    nc.vector.tensor_scalar(idf[:, :], id_i[:, :], 0, None,
                            mybir.AluOpType.is_equal)

    # kcols[s*64+c, t] = kern[s, c*9+t]  (fp32, per-partition tap scalars) and
    # fp16 diagonal weight matrices, built in PE-tap order so the first
    # depthwise matmul can issue as early as possible.
    # each tap's block-diag Wk2 matmul produces kern for all 128 (s,c) pairs
    # as one [128,1] column; the PSUM->SBUF copies run on the (idle) Act
    # engine so DVE can start tap products immediately.
    ntap = KK * KK
    kcols = const.tile([P128, ntap], f32)
    diag = const.tile([P128, ntap * P128], f16)
    tap_order = [PE_TAPS[0]] + DVE_TAPS + DVE_PAIR + PE_TAPS[1:]
    for (di, dj) in tap_order:
        t = di * KK + dj
        kp = po.tile([P128, 1], f32, tag="o")
        nc.tensor.matmul(kp[:, :], lhsT=wk2b[:, t * P128 : (t + 1) * P128],
                         rhs=hid_sb[:, :], start=True, stop=True)
        if (di, dj) in PE_TAPS:
            nc.scalar.activation(kcols[:, t : t + 1], kp[:, :],
                                 mybir.ActivationFunctionType.Copy)
            nc.vector.tensor_scalar_mul(diag[:, t * P128 : (t + 1) * P128],
                                        idf[:, :], kcols[:, t : t + 1])
        else:
            nc.vector.tensor_copy(kcols[:, t : t + 1], kp[:, :])

    # ---------------- main loop ----------------
    xr = xs[:, :].rearrange("p (r w) -> p r w", w=W)

    def kap(di, dj):
        return kcols[:, (di * KK + dj) : (di * KK + dj) + 1]

    def dve_products(k0, nt):
        # tap products for nt consecutive super-tiles in one go -- larger TS
        # ops amortize the per-op DVE init (4x DVE mode). All four products
        # are full-width x-aligned.
        parts = []
        for i, (di, dj) in enumerate(DVE_TAPS + DVE_PAIR):
            t = vpool.tile([P128, nt * SPX], f16, tag=f"t{i}")
            w = xr[:, SR * k0 + di : SR * k0 + di + nt * SR, :]
            nc.vector.tensor_scalar_mul(t[:, :], w, kap(di, dj))
            parts.append(t)
        return parts

    def dve_merge(k, parts, off):
        # per-tile TT merges keep v latency tile-local; the dj=0 pair merges
        # via a column-trimmed TT, realizing SAME padding (col 0 gets no
        # dj=0 contribution).
        t0, t1, t2, t3 = parts
        sl = slice(off, off + SPX)
        v = vpool.tile([P128, SPX], f16, tag="v")
        nc.vector.tensor_tensor(v[:, :], t0[:, sl], t1[:, sl],
                                op=mybir.AluOpType.add)
        nc.vector.tensor_tensor(t2[:, sl], t2[:, sl], t3[:, sl],
                                op=mybir.AluOpType.add)
        vv = v[:, :].rearrange("p (r w) -> p r w", w=W)
        qv = t2[:, sl].rearrange("p (r w) -> p r w", w=W)
        nc.vector.tensor_tensor(vv[:, :, 1:W], vv[:, :, 1:W],
                                qv[:, :, 0 : W - 1], op=mybir.AluOpType.add)
        return v

    def pe_taps_half(k, P, h):
        Pv = P[:, :].rearrange("p (r w) -> p r w", w=W)
        out = Pv[:, 4 * h : 4 * h + 4, :]
        r0 = SR * k + 4 * h
        for i, (di, dj) in enumerate(PE_TAPS):
            ci, co = _cols(dj)
            nc.tensor.matmul(
                out[:, :, co],
                lhsT=diag[:, (di * KK + dj) * P128 : (di * KK + dj + 1) * P128],
                rhs=xr[:, r0 + di : r0 + di + 4, ci],
                start=(i == 0), stop=False,
                skip_group_check=True)

    def pe_iadd_half(k, P, v, h):
        # identity matmul folds the DVE partial into the accumulation group
        # (stop=True): no cross-engine PSUM merge needed.
        nc.tensor.matmul(
            P[:, 512 * h : 512 * (h + 1)], lhsT=idf[:, :],
            rhs=v[:, 512 * h : 512 * (h + 1)],
            start=False, stop=True, skip_group_check=True)

    def pe_taps(k):
        P = pdw.tile([P128, SPX], f32, tag="p")
        for h in (0, 1):
            pe_taps_half(k, P, h)
        return P

    def pe_iadd(k, P, v):
        for h in (0, 1):
            pe_iadd_half(k, P, v, h)

    def prelu_stage(k, P):
        # Full-width in steady state; half granularity in the last tiles
        # keeps the pipeline drain short.
        halves = k >= NST - 2
        a = apool.tile([P128, SPX], f16, tag="a")
        for hs in ([slice(0, 512), slice(512, SPX)] if halves
                   else [slice(0, SPX)]):
            nc.scalar.activation(a[:, hs], P[:, hs], LRELU, alpha=0.1)
        return a

    def conv1x1_half(k, h, a):
        O = po.tile([P128, 512], f32, tag="o")
        nc.tensor.matmul(O[:, :], lhsT=wc2[:, :], rhs=a[:, 512 * h : 512 * (h + 1)],
                         start=True, stop=True)
        z = zpool.tile([P128, 512], f16, tag=f"z{h}")
        if k >= NST - 2:
            # Act is congested with prelu halves during the drain; DVE idles
            nc.vector.tensor_copy(z[:, :], O[:, :])
        else:
            nc.scalar.activation(z[:, :], O[:, :], LRELU, alpha=1.0)
        nc.sync.dma_start(out_d[:, k * SPX + 512 * h : k * SPX + 512 * (h + 1)],
                          z[:, :])

    # DVE product granularity: single tiles at the pipeline ends (short
    # v latency for fill/drain), 4-tile blocks in the middle (lower DVE op
    # overhead)
    blocks = [(0, 1), (1, 1)] + [(k, 2) for k in range(2, NST - 2, 2)] + [
        (NST - 2, 1), (NST - 1, 1)]
    tiles = []
    for (k0, nt) in blocks:
        tiles.extend((k0, nt, i) for i in range(nt))

    pending = []
    parts = None
    for (k0, nt, i) in tiles:
        k = k0 + i
        if i == 0:
            parts = dve_products(k0, nt)
        v = dve_merge(k, parts, SPX * i)
        P = pe_taps(k)
        pe_iadd(k, P, v)
        a = prelu_stage(k, P)
        for item in pending:
            conv1x1_half(*item)
        pending = [(k, 0, a), (k, 1, a)]
    for item in pending:
        conv1x1_half(*item)


# ---------------------------------------------------------------------------
# host-side entry point
# ---------------------------------------------------------------------------

_PROGRAM_CACHE: dict[str, bass.Bass] = {}


def _get_program(x_mode: str = X_MODE) -> bass.Bass:
    if x_mode not in _PROGRAM_CACHE:
        _PROGRAM_CACHE[x_mode] = build_program()
    return _PROGRAM_CACHE[x_mode]


def _host_prep(inputs: dict, x_mode: str = X_MODE):
    x = np.asarray(inputs["x"], dtype=np.float32)
    d = np.asarray(inputs["d"], dtype=np.float32)
    Wk1 = np.asarray(inputs["Wk1"], dtype=np.float32)
    Wk2 = np.asarray(inputs["Wk2"], dtype=np.float32)
    Wc = np.asarray(inputs["Wc"], dtype=np.float32)

    wk1b = np.zeros((S * C, S * C), dtype=np.float32)
    wk1b[0:C, 0:C] = Wk1.T
    wk1b[C:, C:] = Wk1.T

    # wk2b[:, t*128:(t+1)*128] = block-diag(Wk2_t.T, Wk2_t.T),
    # where Wk2_t[c, j] = Wk2[c*9+t, j]
    w = Wk2.reshape(C, KK * KK, C)                    # (c, t, j)
    wk2b = np.zeros((S * C, KK * KK, S * C), dtype=np.float16)
    wk2b[0:C, :, 0:C] = w.transpose(2, 1, 0)          # (j, t, c)
    wk2b[C:, :, C:] = w.transpose(2, 1, 0)
    wk2b = np.ascontiguousarray(
        wk2b.transpose(0, 1, 2).reshape(S * C, KK * KK * S * C)
    )

    wc2 = np.zeros((S * C, S * C), dtype=np.float16)
    wc2[0:C, 0:C] = Wc.T
    wc2[C:, C:] = Wc.T

    xcast = x.astype(np.float16)

    in_maps = []
    for i in range(NCORES):
        xi = np.ascontiguousarray(xcast[S * i : S * (i + 1)].reshape(S * C, H * W))
        d2 = np.ascontiguousarray(
            d[S * i : S * (i + 1)].reshape(S * C, 1)
        )
        in_maps.append(
            {"x": xi, "d2": d2, "wk1b": wk1b, "wk2b": wk2b, "wc2": wc2}
        )
    return in_maps


def run_on_hw(inputs: dict, x_mode: str = X_MODE, **kwargs):
    """Run the SPMD kernel on 8 NeuronCores; returns (output, results)."""
    from concourse.bass_utils import run_bass_kernel_spmd

    nc = _get_program(x_mode)
    in_maps = _host_prep(inputs, x_mode)
    res = run_bass_kernel_spmd(nc, in_maps, core_ids=list(range(NCORES)), **kwargs)
    outs = res.results
    bc = np.asarray(inputs["bc"], dtype=np.float32)
    B = S * NCORES
    out = np.empty((B, C, H, W), dtype=np.float32)
    for i in range(NCORES):
        out[S * i : S * (i + 1)] = (
            outs[i]["out"].astype(np.float32).reshape(S, C, H, W)
        )
    out += bc[None, :, None, None]
    return out, res


def kernel(**inputs) -> np.ndarray:
    out, _ = run_on_hw(inputs)
    return out


if __name__ == "__main__":
    nc = build_program()
    print("program built OK")
